# revision 40
# baseline (speedup 1.0000x reference)
"""Trainium2 Bass kernel: multi-head self-attention (B=2, T=2048, D=1024, H=16),
8-core SPMD. Accepts FULL inputs, returns the FULL output.

Sharding: data-parallel over batch (2) x tensor-parallel over heads (4 groups
of 4). Each core computes attention for its 4 heads of one batch plus its
partial output projection; the host sums the 4 partials per batch (plus the
bias terms, folded exactly).

vE: scalar engine does ONLY exp; all PSUM->SBUF evacuation on DVE; causal
mask via gpsimd affine_select (no mask tensor, no DVE mask pass); exp table
prefetched during the DMA lead-in; 7-group k-interleaved opening wave
consumes x as its DMA lands; deep exp->PV pipeline (LAG=7, 10 pt buffers)
so attention S/exp runs well ahead of PV and the injected QKV/proj work can
fill PE bubbles; per-block projection injected one block early; bf16 output.
Warm-filler matmuls bridge the late norm lulls. Measured 171.0us on HW
(baseline 177.4us); note the device P0-downclocks ~15% when hot.
"""
import sys
if '/opt/trn_rl_repo' not in sys.path:
    sys.path.insert(0, '/opt/trn_rl_repo')
import numpy as np
import ml_dtypes
import concourse.bass as bass
import concourse.mybir as mybir
from concourse import bacc
from concourse.tile import TileContext

F32 = mybir.dt.float32
BF16 = mybir.dt.bfloat16
AL = mybir.AluOpType
EXP = mybir.ActivationFunctionType.Exp
BF = ml_dtypes.bfloat16

T = 2048
DM = 1024
HPC = 4
D = 64
NQB = 4           # query blocks of 512
NKC = 16          # key chunks of 128
NDC = 8           # contraction chunks of 128 for projections
LAG = 7           # PV lags S/exp by this many key chunks
WARMUP = 64       # contiguous warmup matmuls bridging the DMA lead-in


def build_nc():
    nc = bacc.Bacc("TRN2", target_bir_lowering=False, debug=True)

    xp = nc.dram_tensor("xp", [128, NDC, T], BF16, kind="ExternalInput")
    wqk = nc.dram_tensor("wqk", [128, NDC, 512], BF16, kind="ExternalInput")
    wv = nc.dram_tensor("wv", [128, NDC, 260], BF16, kind="ExternalInput")
    wp = nc.dram_tensor("wp", [128, 2, DM], BF16, kind="ExternalInput")
    y = nc.dram_tensor("y", [T, DM], BF16, kind="ExternalOutput")

    with nc.allow_low_precision("bf16 matmul pipeline"), TileContext(nc) as tc:
        from contextlib import ExitStack
        ctx = ExitStack()
        cp = ctx.enter_context(tc.tile_pool(name="const", bufs=1))
        wtp = ctx.enter_context(tc.tile_pool(name="wts", bufs=1))
        qkvp = ctx.enter_context(tc.tile_pool(name="qkv", bufs=1))
        psS = ctx.enter_context(tc.tile_pool(name="psS", bufs=2, space="PSUM"))
        psO = ctx.enter_context(tc.tile_pool(name="psO", bufs=1, space="PSUM"))
        psX = ctx.enter_context(tc.tile_pool(name="psX", bufs=2, space="PSUM"))

        wqk_t = wtp.tile([128, NDC * 512], BF16, tag="wqk", name="wqk")
        wv_t = wtp.tile([128, NDC * 260], BF16, tag="wv", name="wv")
        wp_t = wtp.tile([128, 2 * DM], BF16, tag="wp", name="wp")
        ones_t = cp.tile([128, 64], BF16, tag="ones", name="ones")
        nc.vector.memset(ones_t[:], 1.0)
        onesv = cp.tile([128, 4], BF16, tag="onesv", name="onesv")
        nc.vector.memset(onesv[:], 1.0)
        # prefetch the exp table set (~2.7us) during the DMA lead-in so the
        # first real attention exp doesn't pay it
        escr = cp.tile([1, 64], BF16, tag="escr", name="escr")
        nc.scalar.activation(escr[:], ones_t[0:1, 0:64], EXP)

        # persistent activations
        QT = [qkvp.tile([128, T], BF16, tag=f"qt{i}", name=f"qt{i}") for i in range(2)]
        KT = [qkvp.tile([128, T], BF16, tag=f"kt{i}", name=f"kt{i}") for i in range(2)]
        V = [qkvp.tile([128, 260], BF16, tag=f"v{t}", name=f"v{t}") for t in range(NKC)]
        OTS = [qkvp.tile([128, T], BF16, tag=f"ots{j}", name=f"ots{j}")
               for j in range(2)]

        def wq_sl(k, fc):
            return wqk_t[:, k * 512 + fc * 128:k * 512 + fc * 128 + 128]

        def wk_sl(k, fc):
            return wqk_t[:, k * 512 + 256 + fc * 128:k * 512 + 256 + fc * 128 + 128]

        def wv_sl(k):
            return wv_t[:, k * 260:(k + 1) * 260]

        def wp_sl(jc, mb):
            return wp_t[:, jc * DM + mb * 512:jc * DM + mb * 512 + 512]

        # ---------------- phase B: QKV projections ----------------
        xtp = ctx.enter_context(tc.tile_pool(name="xt", bufs=1))
        xt2 = [xtp.tile([128, 2 * T], BF16, tag=f"xa{i}", name=f"xa{i}")
               for i in range(4)]

        def xt_sl(k, c0, c1):
            return xt2[k // 2][:, (k % 2) * T + c0:(k % 2) * T + c1]

        # warmup: keep the PE busy (and HAM un-throttled) across the input
        # DMA lead-in; sized so real matmuls aren't head-of-line blocked
        wps = psX.tile([64, 512], F32, tag="b", name="warm")
        for i in range(WARMUP):
            nc.tensor.matmul(wps[:, 0:64], ones_t[0:64, 0:64],
                             ones_t[0:64, 0:64], start=True, stop=True)
        nc.vector.tensor_copy(ones_t[:], ones_t[:])  # keep wps unread harmless
        nc.sync.dma_start(wqk_t[:, 0:4 * 512], wqk[:, 0:4, :])
        nc.sync.dma_start(wv_t[:, 0:4 * 260], wv[:, 0:4, :])
        nc.sync.dma_start(xt2[0][:], xp[:, 0:2, :])
        nc.sync.dma_start(xt2[1][:], xp[:, 2:4, :])
        nc.sync.dma_start(wqk_t[:, 4 * 512:], wqk[:, 4:8, :])
        nc.sync.dma_start(wv_t[:, 4 * 260:], wv[:, 4:8, :])
        nc.sync.dma_start(xt2[2][:], xp[:, 4:6, :])
        nc.sync.dma_start(xt2[3][:], xp[:, 6:8, :])
        nc.sync.dma_start(wp_t[:], wp[:, :, :])

        def v_finish(tt, ps_ap):
            nc.vector.tensor_copy(V[tt][:], ps_ap)
            nc.vector.tensor_copy(
                V[tt].rearrange("p (h c) -> p h c", c=65)[:, :, 64:65],
                onesv[:].rearrange("p (h c) -> p h c", c=1))

        def qk_group(is_k, fc, tb):
            OUT = KT if is_k else QT
            ps = psX.tile([128, 512], F32, tag="b", name="qkps")
            for k in range(NDC):
                w = wk_sl(k, fc) if is_k else wq_sl(k, fc)
                nc.tensor.matmul(
                    ps[:], w, xt_sl(k, tb * 512, (tb + 1) * 512),
                    start=(k == 0), stop=(k == NDC - 1))
            nc.vector.tensor_copy(OUT[fc][:, tb * 512:(tb + 1) * 512], ps[:])

        def v_tile(tt):
            ps = psX.tile([128, 260], F32, tag="b", name="vps")
            for k in range(NDC):
                nc.tensor.matmul(
                    ps[:], xt_sl(k, tt * 128, (tt + 1) * 128), wv_sl(k),
                    start=(k == 0), stop=(k == NDC - 1))
            v_finish(tt, ps)

        # opening wave: six PSUM groups k-interleaved so the PE consumes each
        # x chunk-pair as its DMA lands (Q/K block 0 borrow the idle psS ring)
        psq0 = psS.tile([128, 1024], F32, tag="s", name="s")
        psq1 = psS.tile([128, 1024], F32, tag="s", name="s")
        psv0 = psX.tile([128, 260], F32, tag="b", name="vps")
        psv1 = psX.tile([128, 260], F32, tag="b", name="vps")
        wps2 = psO.tile([64, 512], F32, tag="o0", name="warm2")
        psq2 = psO.tile([128, 512], F32, tag="o1", name="q01")
        for k in range(NDC):
            nc.tensor.matmul(psq0[:, 0:512], wq_sl(k, 0),
                             xt_sl(k, 0, 512),
                             start=(k == 0), stop=(k == NDC - 1))
            nc.tensor.matmul(psq0[:, 512:1024], wk_sl(k, 0),
                             xt_sl(k, 0, 512),
                             start=(k == 0), stop=(k == NDC - 1))
            nc.tensor.matmul(psv0[:], xt_sl(k, 0, 128), wv_sl(k),
                             start=(k == 0), stop=(k == NDC - 1))
            nc.tensor.matmul(psv1[:], xt_sl(k, 128, 256), wv_sl(k),
                             start=(k == 0), stop=(k == NDC - 1))
            nc.tensor.matmul(psq1[:, 0:260], xt_sl(k, 256, 384), wv_sl(k),
                             start=(k == 0), stop=(k == NDC - 1))
            nc.tensor.matmul(psq1[:, 512:772], xt_sl(k, 384, 512), wv_sl(k),
                             start=(k == 0), stop=(k == NDC - 1))
            nc.tensor.matmul(psq2[:], wq_sl(k, 0), xt_sl(k, 512, 1024),
                             start=(k == 0), stop=(k == NDC - 1))
        nc.vector.tensor_copy(QT[0][:, 0:512], psq0[:, 0:512])
        nc.vector.tensor_copy(KT[0][:, 0:512], psq0[:, 512:1024])
        nc.vector.tensor_copy(QT[0][:, 512:1024], psq2[:])
        for tt, psv in ((0, psv0), (1, psv1)):
            v_finish(tt, psv[:])
        v_finish(2, psq1[:, 0:260])
        v_finish(3, psq1[:, 512:772])

        # ---------------- phase C: attention ----------------
        ptp = ctx.enter_context(tc.tile_pool(name="pt", bufs=10))
        rcp = ctx.enter_context(tc.tile_pool(name="rcp", bufs=2))
        ybp = ctx.enter_context(tc.tile_pool(name="yb", bufs=4))

        dd_all = {}   # (hp, hh, qb) -> [1, 512] f32 reciprocal denominators
        ou_all = {}   # (hp, hh, qb) -> [65, 512] f32 unnormalized O (+denom)

        def norm_emit(hp, hh, qb):
            """Normalize O from its SBUF copy into OTS (hh=1 via DMA bounce
            for the partition shift)."""
            ou = ou_all[(hp, hh, qb)]
            ddr = dd_all[(hp, hh, qb)]
            dnb = rcp.tile([64, 512], F32, tag="dnb", name="dnb", bufs=3)
            nc.gpsimd.partition_broadcast(dnb[:], ddr[0:1, :], channels=64)
            if hh == 0:
                nc.vector.tensor_tensor(
                    OTS[hp][0:64, qb * 512:(qb + 1) * 512],
                    ou[0:64, :], dnb[:], AL.mult)
            else:
                # NOTE: keep this on vector — gpsimd tensor_tensor lives in a
                # different DSP library than affine_select/broadcast and every
                # use forces a ~6us UNLOAD_LIB/LOAD_LIB swap
                ob = rcp.tile([64, 512], BF16, tag="ob",
                              name="ob", bufs=3)
                nc.vector.tensor_tensor(
                    ob[:], ou[0:64, :], dnb[:], AL.mult)
                nc.sync.dma_start(
                    OTS[hp][64:128, qb * 512:(qb + 1) * 512], ob[:])

        ybt = {}

        def proj_unit(tt, mb):
            def emit():
                pool = psX if (tt + mb) % 2 == 0 else psS
                psy = pool.tile([128, 512], F32, tag="b" if pool is psX else "s",
                                name="yps")
                for jc in range(2):
                    nc.tensor.matmul(
                        psy[:], OTS[jc][:, tt * 128:(tt + 1) * 128],
                        wp_sl(jc, mb), start=(jc == 0), stop=(jc == 1))
                yt = ybp.tile([128, 512], BF16, tag="yt", name="yt")
                nc.vector.tensor_copy(yt[:], psy[:])
                nc.sync.dma_start(
                    y[tt * 128:(tt + 1) * 128, mb * 512:(mb + 1) * 512],
                    yt[:])
            return emit

        # deadline-ordered injections for hp0: remaining fc0 Q/K blocks
        # (block tb needed when query/key block tb starts), V tiles (tile kc
        # needed at PV chunk kc), then the fc1 groups for hp1
        def QG(is_k, fc, g):
            return lambda: qk_group(is_k, fc, g)

        def VT(tt):
            return lambda: v_tile(tt)

        inj0 = [QG(True, 0, 1),
                VT(4), VT(5), QG(False, 0, 2), VT(6), VT(7),
                QG(True, 0, 2), VT(8), QG(False, 0, 3), VT(9),
                QG(True, 0, 3), VT(10), VT(11),
                QG(False, 1, 0), VT(12), QG(False, 1, 1), VT(13),
                QG(False, 1, 2), VT(14), QG(False, 1, 3), VT(15),
                QG(True, 1, 0), QG(True, 1, 1), QG(True, 1, 2),
                QG(True, 1, 3)]
        inject = list(inj0)

        for hp in range(HPC // 2):
            fc = hp
            heads = (2 * hp, 2 * hp + 1)
            qb_order = list(range(NQB)) if hp == 0 else [3, 2, 1, 0]
            for qb in qb_order:
                nkc = 4 * (qb + 1)
                pso = {h: psO.tile([65, 512], F32, tag=f"o{h % 2}",
                                   name=f"o{h % 2}") for h in heads}
                ptq = {}
                offs = {}
                for kc in range(nkc + LAG):
                    if kc < nkc:
                        t = kc - 4 * qb
                        off = 128 * t if t > 0 else 0
                        w = 512 - off
                        pss = psS.tile([128, 1024], F32, tag="s", name="s")
                        for h in heads:
                            po = 64 * (h % 2)
                            nc.tensor.matmul(
                                pss[:, po * 8 + off:po * 8 + 512],
                                KT[fc][po:po + 64, kc * 128:(kc + 1) * 128],
                                QT[fc][po:po + 64,
                                       qb * 512 + off:(qb + 1) * 512],
                                start=True, stop=True)
                        pt = ptp.tile([128, 1024], BF16, tag="pt", name="pt")
                        if off == 0:
                            nc.scalar.activation(pt[:], pss[:], EXP)
                        else:
                            nc.scalar.activation(
                                pt[:].rearrange("p (h q) -> p h q", h=2)[:, :, off:],
                                pss[:].rearrange("p (h q) -> p h q", h=2)[:, :, off:],
                                EXP)
                        if t >= 0:  # diagonal chunk -> causal mask (gpsimd)
                            nc.gpsimd.affine_select(
                                out=pt[:].rearrange(
                                    "p (h q) -> p h q", h=2)[:, :, off:],
                                in_=pt[:].rearrange(
                                    "p (h q) -> p h q", h=2)[:, :, off:],
                                pattern=[[0, 2], [1, w]],
                                compare_op=AL.is_ge,
                                fill=0.0,
                                base=0,
                                channel_multiplier=-1)
                        ptq[kc] = pt
                        offs[kc] = off
                    kcp = kc - LAG
                    if kcp >= 0 and kcp in ptq:
                        ptv = ptq.pop(kcp)
                        off2 = offs.pop(kcp)
                        for h in heads:
                            po = 64 * (h % 2)
                            nc.tensor.matmul(
                                pso[h][:, off2:],
                                V[kcp][:, 65 * h:65 * h + 65],
                                ptv[:, po * 8 + off2:po * 8 + 512],
                                start=(kcp == 0),
                                stop=(kcp == nkc - 1),
                                skip_group_check=True)
                    if kcp >= 0:
                        npop = 1 if hp == 0 else (3 if len(inject) >= 12 else 2)
                        for _ in range(npop):
                            if inject:
                                inject.pop(0)()
                # query-block epilogue: reciprocal of the denominator row
                # straight out of each PV PSUM tile, bounce to partition 0,
                # then broadcast+normalize (norm must finish before the next
                # qb's PV reuses the psO banks)
                # evacuate both PV tiles first (frees the psO banks so the
                # next block's PV can't head-of-line block the PE queue),
                # then run the reciprocal chain from SBUF off-critical-path
                for h in heads:
                    hh = h % 2
                    ou = rcp.tile([65, 512], F32, tag=f"ou{hh}",
                                  name=f"ou{hh}", bufs=3)
                    nc.vector.tensor_copy(ou[:], pso[h][:])
                    ou_all[(hp, hh, qb)] = ou
                for h in heads:
                    hh = h % 2
                    dd = rcp.tile([1, 512], F32, tag=f"dd{hh}",
                                  name=f"dd{hh}", bufs=2)
                    nc.sync.dma_start(dd[:], ou_all[(hp, hh, qb)][64:65, :])
                    ddr = rcp.tile([1, 512], F32, tag=f"ddr{hh}",
                                   name=f"ddr{hh}", bufs=2)
                    nc.vector.reciprocal_approx_fast(ddr[:], dd[:])
                    dd_all[(hp, hh, qb)] = ddr
                for hh in (0, 1):
                    norm_emit(hp, hh, qb)
                if hp == 1:
                    # this block's projection is gated on the norm chain just
                    # emitted; inject it now so it drains during the NEXT
                    # block's attention instead of piling up at the end
                    inject += [proj_unit(tt, mb)
                               for tt in range(4 * qb, 4 * qb + 4)
                               for mb in (0, 1)]
                    if qb <= 1:
                        # warm-filler: the late norm chains idle the PE long
                        # enough to re-throttle the clock; these matmuls are
                        # WAR-gated behind this block's PV evacuation so they
                        # fill exactly the lull and keep the final projections
                        # at full rate
                        wps3 = psO.tile([64, 512], F32, tag="o0",
                                        name=f"warm3_{qb}")
                        for i in range(48 if qb == 1 else 144):
                            nc.tensor.matmul(
                                wps3[:, 0:64], ones_t[0:64, 0:64],
                                ones_t[0:64, 0:64], start=True, stop=True)
        # drain: the final block's projection
        while inject:
            inject.pop(0)()
        ctx.close()

    nc.finalize()
    return nc


def _pack(a, inner):
    """[1024, inner] -> [128, 8, inner] with [p, k, :] = a[k*128+p, :]."""
    return np.ascontiguousarray(
        a.reshape(NDC, 128, inner).transpose(1, 0, 2))


def shard_inputs(x, Wqkv, bqkv, Wproj):
    x = np.asarray(x, dtype=np.float32)
    Wqkv = np.asarray(Wqkv, dtype=np.float32)
    bqkv = np.asarray(bqkv, dtype=np.float32)
    Wproj = np.asarray(Wproj, dtype=np.float32)
    assert not np.any(bqkv[0:2048]), \
        "nonzero q/k bias not supported by the fast kernel"
    in_maps = []
    for c in range(8):
        b, g = c // 4, c % 4
        cs = slice(256 * g, 256 * g + 256)
        wq_ = Wqkv[:, 0:1024][:, cs] / 8.0
        wk_ = Wqkv[:, 1024:2048][:, cs]
        wqk_ = np.concatenate([wq_, wk_], axis=1)  # [1024, 512]
        wv_src = Wqkv[:, 2048:3072][:, cs]
        wv_ = np.zeros((DM, 260), dtype=np.float32)
        for h in range(4):
            wv_[:, 65 * h:65 * h + 64] = wv_src[:, 64 * h:64 * h + 64]
        wp_ = np.ascontiguousarray(
            Wproj[256 * g:256 * g + 256, :].reshape(2, 128, DM)
            .transpose(1, 0, 2))
        in_maps.append({
            "xp": _pack(x[b].T, T).astype(BF),
            "wqk": _pack(wqk_, 512).astype(BF),
            "wv": _pack(wv_, 260).astype(BF),
            "wp": wp_.astype(BF),
        })
    return in_maps


def combine_outputs(results, Wqkv, bqkv, Wproj, bproj):
    bqkv = np.asarray(bqkv, dtype=np.float32)
    Wproj = np.asarray(Wproj, dtype=np.float32)
    bproj = np.asarray(bproj, dtype=np.float32)
    bv_term = bqkv[2048:3072] @ Wproj
    out = np.zeros((2, T, DM), dtype=np.float32)
    for c in range(8):
        out[c // 4] += results[c]["y"].astype(np.float32).reshape(T, DM)
    out += (bv_term + bproj)[None, None, :]
    return out


_NC_CACHE = []


def _numpy_fallback(x, Wqkv, bqkv, Wproj, bproj):
    b, t, dm = x.shape
    h, d = 16, 64
    qkv = x @ Wqkv + bqkv
    q, k, v = np.split(qkv, 3, axis=-1)
    q = q.reshape(b, t, h, d).transpose(0, 2, 1, 3)
    k = k.reshape(b, t, h, d).transpose(0, 2, 1, 3)
    v = v.reshape(b, t, h, d).transpose(0, 2, 1, 3)
    att = np.einsum('bhqd,bhkd->bhqk', q, k) / np.sqrt(np.float32(d))
    causal = np.tril(np.ones((t, t), dtype=bool))
    att = np.where(causal[None, None], att, -np.inf)
    att = att - att.max(axis=-1, keepdims=True)
    e = np.exp(att)
    p = e / e.sum(axis=-1, keepdims=True)
    out = np.einsum('bhqk,bhkd->bhqd', p, v)
    out = out.transpose(0, 2, 1, 3).reshape(b, t, dm)
    return (out @ Wproj + bproj).astype(np.float32)


def kernel(x, Wqkv, bqkv, Wproj, bproj):
    x = np.asarray(x, dtype=np.float32)
    Wqkv = np.asarray(Wqkv, dtype=np.float32)
    bqkv = np.asarray(bqkv, dtype=np.float32)
    Wproj = np.asarray(Wproj, dtype=np.float32)
    bproj = np.asarray(bproj, dtype=np.float32)
    if np.any(bqkv[0:2048]):
        return _numpy_fallback(x, Wqkv, bqkv, Wproj, bproj)
    from concourse.bass_utils import run_bass_kernel_spmd
    if not _NC_CACHE:
        _NC_CACHE.append(build_nc())
    nc = _NC_CACHE[0]
    in_maps = shard_inputs(x, Wqkv, bqkv, Wproj)
    res = run_bass_kernel_spmd(nc, in_maps, core_ids=list(range(8)))
    return combine_outputs(res.results, Wqkv, bqkv, Wproj, bproj)


# revision 41
# speedup vs baseline: 1.0051x; 1.0051x over previous
"""Trainium2 Bass kernel: multi-head self-attention (B=2, T=2048, D=1024, H=16),
8-core SPMD. Accepts FULL inputs, returns the FULL output.

Sharding: data-parallel over batch (2) x tensor-parallel over heads (4 groups
of 4). Each core computes attention for its 4 heads of one batch plus its
partial output projection; the host sums the 4 partials per batch (plus the
bias terms, folded exactly).

vE: scalar engine does ONLY exp; all PSUM->SBUF evacuation on DVE; causal
mask via gpsimd affine_select (no mask tensor, no DVE mask pass); exp table
prefetched during the DMA lead-in; 7-group k-interleaved opening wave
consumes x as its DMA lands; deep exp->PV pipeline (LAG=7, 10 pt buffers)
so attention S/exp runs well ahead of PV and the injected QKV/proj work can
fill PE bubbles; per-block projection injected one block early; bf16 output.
Warm-filler matmuls bridge the late norm lulls. Measured 171.0us on HW
(baseline 177.4us); note the device P0-downclocks ~15% when hot.
"""
import sys
if '/opt/trn_rl_repo' not in sys.path:
    sys.path.insert(0, '/opt/trn_rl_repo')
import numpy as np
import ml_dtypes
import concourse.bass as bass
import concourse.mybir as mybir
from concourse import bacc
from concourse.tile import TileContext

F32 = mybir.dt.float32
BF16 = mybir.dt.bfloat16
AL = mybir.AluOpType
EXP = mybir.ActivationFunctionType.Exp
BF = ml_dtypes.bfloat16

T = 2048
DM = 1024
HPC = 4
D = 64
NQB = 4           # query blocks of 512
NKC = 16          # key chunks of 128
NDC = 8           # contraction chunks of 128 for projections
LAG = 7           # PV lags S/exp by this many key chunks
WARMUP = 64       # contiguous warmup matmuls bridging the DMA lead-in


def build_nc():
    nc = bacc.Bacc("TRN2", target_bir_lowering=False, debug=True)

    xp = nc.dram_tensor("xp", [128, NDC, T], BF16, kind="ExternalInput")
    wqk = nc.dram_tensor("wqk", [128, NDC, 512], BF16, kind="ExternalInput")
    wv = nc.dram_tensor("wv", [128, NDC, 260], BF16, kind="ExternalInput")
    wp = nc.dram_tensor("wp", [128, 2, DM], BF16, kind="ExternalInput")
    y = nc.dram_tensor("y", [T, DM], BF16, kind="ExternalOutput")

    with nc.allow_low_precision("bf16 matmul pipeline"), TileContext(nc) as tc:
        from contextlib import ExitStack
        ctx = ExitStack()
        cp = ctx.enter_context(tc.tile_pool(name="const", bufs=1))
        wtp = ctx.enter_context(tc.tile_pool(name="wts", bufs=1))
        qkvp = ctx.enter_context(tc.tile_pool(name="qkv", bufs=1))
        psS = ctx.enter_context(tc.tile_pool(name="psS", bufs=2, space="PSUM"))
        psO = ctx.enter_context(tc.tile_pool(name="psO", bufs=1, space="PSUM"))
        psX = ctx.enter_context(tc.tile_pool(name="psX", bufs=2, space="PSUM"))

        wqk_t = wtp.tile([128, NDC * 512], BF16, tag="wqk", name="wqk")
        wv_t = wtp.tile([128, NDC * 260], BF16, tag="wv", name="wv")
        wp_t = wtp.tile([128, 2 * DM], BF16, tag="wp", name="wp")
        ones_t = cp.tile([128, 64], BF16, tag="ones", name="ones")
        nc.vector.memset(ones_t[:], 1.0)
        onesv = cp.tile([128, 4], BF16, tag="onesv", name="onesv")
        nc.vector.memset(onesv[:], 1.0)
        # prefetch the exp table set (~2.7us) during the DMA lead-in so the
        # first real attention exp doesn't pay it
        escr = cp.tile([1, 64], BF16, tag="escr", name="escr")
        nc.scalar.activation(escr[:], ones_t[0:1, 0:64], EXP)

        # persistent activations
        QT = [qkvp.tile([128, T], BF16, tag=f"qt{i}", name=f"qt{i}") for i in range(2)]
        KT = [qkvp.tile([128, T], BF16, tag=f"kt{i}", name=f"kt{i}") for i in range(2)]
        V = [qkvp.tile([128, 260], BF16, tag=f"v{t}", name=f"v{t}") for t in range(NKC)]
        OTS = [qkvp.tile([128, T], BF16, tag=f"ots{j}", name=f"ots{j}")
               for j in range(2)]

        def wq_sl(k, fc):
            return wqk_t[:, k * 512 + fc * 128:k * 512 + fc * 128 + 128]

        def wk_sl(k, fc):
            return wqk_t[:, k * 512 + 256 + fc * 128:k * 512 + 256 + fc * 128 + 128]

        def wv_sl(k):
            return wv_t[:, k * 260:(k + 1) * 260]

        def wp_sl(jc, mb):
            return wp_t[:, jc * DM + mb * 512:jc * DM + mb * 512 + 512]

        # ---------------- phase B: QKV projections ----------------
        xtp = ctx.enter_context(tc.tile_pool(name="xt", bufs=1))
        xt2 = [xtp.tile([128, 2 * T], BF16, tag=f"xa{i}", name=f"xa{i}")
               for i in range(4)]

        def xt_sl(k, c0, c1):
            return xt2[k // 2][:, (k % 2) * T + c0:(k % 2) * T + c1]

        # warmup: keep the PE busy (and HAM un-throttled) across the input
        # DMA lead-in; sized so real matmuls aren't head-of-line blocked
        wps = psX.tile([64, 512], F32, tag="b", name="warm")
        for i in range(WARMUP):
            nc.tensor.matmul(wps[:, 0:64], ones_t[0:64, 0:64],
                             ones_t[0:64, 0:64], start=True, stop=True)
        nc.vector.tensor_copy(ones_t[:], ones_t[:])  # keep wps unread harmless
        # x is issued per k-chunk (halves of each xt2 tile) so the opening
        # wave's chunk-k matmuls unblock as soon as their own 0.5MB lands
        nc.sync.dma_start(wqk_t[:, 0:4 * 512], wqk[:, 0:4, :])
        nc.sync.dma_start(wv_t[:, 0:4 * 260], wv[:, 0:4, :])
        for k in range(NDC):
            nc.sync.dma_start(xt2[k // 2][:, (k % 2) * T:(k % 2) * T + T],
                              xp[:, k:k + 1, :])
            if k == 3:
                nc.sync.dma_start(wqk_t[:, 4 * 512:], wqk[:, 4:8, :])
                nc.sync.dma_start(wv_t[:, 4 * 260:], wv[:, 4:8, :])
        nc.sync.dma_start(wp_t[:], wp[:, :, :])

        def v_finish(tt, ps_ap):
            nc.vector.tensor_copy(V[tt][:], ps_ap)
            nc.vector.tensor_copy(
                V[tt].rearrange("p (h c) -> p h c", c=65)[:, :, 64:65],
                onesv[:].rearrange("p (h c) -> p h c", c=1))

        def qk_group(is_k, fc, tb):
            OUT = KT if is_k else QT
            ps = psX.tile([128, 512], F32, tag="b", name="qkps")
            for k in range(NDC):
                w = wk_sl(k, fc) if is_k else wq_sl(k, fc)
                nc.tensor.matmul(
                    ps[:], w, xt_sl(k, tb * 512, (tb + 1) * 512),
                    start=(k == 0), stop=(k == NDC - 1))
            nc.vector.tensor_copy(OUT[fc][:, tb * 512:(tb + 1) * 512], ps[:])

        def v_tile(tt):
            ps = psX.tile([128, 260], F32, tag="b", name="vps")
            for k in range(NDC):
                nc.tensor.matmul(
                    ps[:], xt_sl(k, tt * 128, (tt + 1) * 128), wv_sl(k),
                    start=(k == 0), stop=(k == NDC - 1))
            v_finish(tt, ps)

        # opening wave: six PSUM groups k-interleaved so the PE consumes each
        # x chunk-pair as its DMA lands (Q/K block 0 borrow the idle psS ring)
        psq0 = psS.tile([128, 1024], F32, tag="s", name="s")
        psq1 = psS.tile([128, 1024], F32, tag="s", name="s")
        psv0 = psX.tile([128, 260], F32, tag="b", name="vps")
        psv1 = psX.tile([128, 260], F32, tag="b", name="vps")
        wps2 = psO.tile([64, 512], F32, tag="o0", name="warm2")
        psq2 = psO.tile([128, 512], F32, tag="o1", name="q01")
        for k in range(NDC):
            nc.tensor.matmul(psq0[:, 0:512], wq_sl(k, 0),
                             xt_sl(k, 0, 512),
                             start=(k == 0), stop=(k == NDC - 1))
            nc.tensor.matmul(psq0[:, 512:1024], wk_sl(k, 0),
                             xt_sl(k, 0, 512),
                             start=(k == 0), stop=(k == NDC - 1))
            nc.tensor.matmul(psv0[:], xt_sl(k, 0, 128), wv_sl(k),
                             start=(k == 0), stop=(k == NDC - 1))
            nc.tensor.matmul(psv1[:], xt_sl(k, 128, 256), wv_sl(k),
                             start=(k == 0), stop=(k == NDC - 1))
            nc.tensor.matmul(psq1[:, 0:260], xt_sl(k, 256, 384), wv_sl(k),
                             start=(k == 0), stop=(k == NDC - 1))
            nc.tensor.matmul(psq1[:, 512:772], xt_sl(k, 384, 512), wv_sl(k),
                             start=(k == 0), stop=(k == NDC - 1))
            nc.tensor.matmul(psq2[:], wq_sl(k, 0), xt_sl(k, 512, 1024),
                             start=(k == 0), stop=(k == NDC - 1))
        nc.vector.tensor_copy(QT[0][:, 0:512], psq0[:, 0:512])
        nc.vector.tensor_copy(KT[0][:, 0:512], psq0[:, 512:1024])
        nc.vector.tensor_copy(QT[0][:, 512:1024], psq2[:])
        for tt, psv in ((0, psv0), (1, psv1)):
            v_finish(tt, psv[:])
        v_finish(2, psq1[:, 0:260])
        v_finish(3, psq1[:, 512:772])

        # ---------------- phase C: attention ----------------
        ptp = ctx.enter_context(tc.tile_pool(name="pt", bufs=10))
        rcp = ctx.enter_context(tc.tile_pool(name="rcp", bufs=2))
        ybp = ctx.enter_context(tc.tile_pool(name="yb", bufs=4))

        dd_all = {}   # (hp, hh, qb) -> [1, 512] f32 reciprocal denominators
        ou_all = {}   # (hp, hh, qb) -> [65, 512] f32 unnormalized O (+denom)

        def norm_emit(hp, hh, qb):
            """Normalize O from its SBUF copy into OTS (hh=1 via DMA bounce
            for the partition shift)."""
            ou = ou_all[(hp, hh, qb)]
            ddr = dd_all[(hp, hh, qb)]
            dnb = rcp.tile([64, 512], F32, tag="dnb", name="dnb", bufs=3)
            nc.gpsimd.partition_broadcast(dnb[:], ddr[0:1, :], channels=64)
            if hh == 0:
                nc.vector.tensor_tensor(
                    OTS[hp][0:64, qb * 512:(qb + 1) * 512],
                    ou[0:64, :], dnb[:], AL.mult)
            else:
                # NOTE: keep this on vector — gpsimd tensor_tensor lives in a
                # different DSP library than affine_select/broadcast and every
                # use forces a ~6us UNLOAD_LIB/LOAD_LIB swap
                ob = rcp.tile([64, 512], BF16, tag="ob",
                              name="ob", bufs=3)
                nc.vector.tensor_tensor(
                    ob[:], ou[0:64, :], dnb[:], AL.mult)
                nc.sync.dma_start(
                    OTS[hp][64:128, qb * 512:(qb + 1) * 512], ob[:])

        ybt = {}

        def proj_unit(tt, mb):
            def emit():
                pool = psX if (tt + mb) % 2 == 0 else psS
                psy = pool.tile([128, 512], F32, tag="b" if pool is psX else "s",
                                name="yps")
                for jc in range(2):
                    nc.tensor.matmul(
                        psy[:], OTS[jc][:, tt * 128:(tt + 1) * 128],
                        wp_sl(jc, mb), start=(jc == 0), stop=(jc == 1))
                yt = ybp.tile([128, 512], BF16, tag="yt", name="yt")
                nc.vector.tensor_copy(yt[:], psy[:])
                nc.sync.dma_start(
                    y[tt * 128:(tt + 1) * 128, mb * 512:(mb + 1) * 512],
                    yt[:])
            return emit

        # deadline-ordered injections for hp0: remaining fc0 Q/K blocks
        # (block tb needed when query/key block tb starts), V tiles (tile kc
        # needed at PV chunk kc), then the fc1 groups for hp1
        def QG(is_k, fc, g):
            return lambda: qk_group(is_k, fc, g)

        def VT(tt):
            return lambda: v_tile(tt)

        inj0 = [QG(True, 0, 1),
                VT(4), VT(5), QG(False, 0, 2), VT(6), VT(7),
                QG(True, 0, 2), VT(8), QG(False, 0, 3), VT(9),
                QG(True, 0, 3), VT(10), VT(11),
                QG(False, 1, 0), VT(12), QG(False, 1, 1), VT(13),
                QG(False, 1, 2), VT(14), QG(False, 1, 3), VT(15),
                QG(True, 1, 0), QG(True, 1, 1), QG(True, 1, 2),
                QG(True, 1, 3)]
        inject = list(inj0)

        for hp in range(HPC // 2):
            fc = hp
            heads = (2 * hp, 2 * hp + 1)
            qb_order = list(range(NQB)) if hp == 0 else [3, 2, 1, 0]
            for qb in qb_order:
                nkc = 4 * (qb + 1)
                pso = {h: psO.tile([65, 512], F32, tag=f"o{h % 2}",
                                   name=f"o{h % 2}") for h in heads}
                ptq = {}
                offs = {}
                for kc in range(nkc + LAG):
                    if kc < nkc:
                        t = kc - 4 * qb
                        off = 128 * t if t > 0 else 0
                        w = 512 - off
                        pss = psS.tile([128, 1024], F32, tag="s", name="s")
                        for h in heads:
                            po = 64 * (h % 2)
                            nc.tensor.matmul(
                                pss[:, po * 8 + off:po * 8 + 512],
                                KT[fc][po:po + 64, kc * 128:(kc + 1) * 128],
                                QT[fc][po:po + 64,
                                       qb * 512 + off:(qb + 1) * 512],
                                start=True, stop=True)
                        pt = ptp.tile([128, 1024], BF16, tag="pt", name="pt")
                        if off == 0:
                            nc.scalar.activation(pt[:], pss[:], EXP)
                        else:
                            nc.scalar.activation(
                                pt[:].rearrange("p (h q) -> p h q", h=2)[:, :, off:],
                                pss[:].rearrange("p (h q) -> p h q", h=2)[:, :, off:],
                                EXP)
                        if t >= 0:  # diagonal chunk -> causal mask (gpsimd)
                            nc.gpsimd.affine_select(
                                out=pt[:].rearrange(
                                    "p (h q) -> p h q", h=2)[:, :, off:],
                                in_=pt[:].rearrange(
                                    "p (h q) -> p h q", h=2)[:, :, off:],
                                pattern=[[0, 2], [1, w]],
                                compare_op=AL.is_ge,
                                fill=0.0,
                                base=0,
                                channel_multiplier=-1)
                        ptq[kc] = pt
                        offs[kc] = off
                    kcp = kc - LAG
                    if kcp >= 0 and kcp in ptq:
                        ptv = ptq.pop(kcp)
                        off2 = offs.pop(kcp)
                        for h in heads:
                            po = 64 * (h % 2)
                            nc.tensor.matmul(
                                pso[h][:, off2:],
                                V[kcp][:, 65 * h:65 * h + 65],
                                ptv[:, po * 8 + off2:po * 8 + 512],
                                start=(kcp == 0),
                                stop=(kcp == nkc - 1),
                                skip_group_check=True)
                    if kcp >= 0:
                        npop = 1 if hp == 0 else (3 if len(inject) >= 12 else 2)
                        for _ in range(npop):
                            if inject:
                                inject.pop(0)()
                # query-block epilogue: reciprocal of the denominator row
                # straight out of each PV PSUM tile, bounce to partition 0,
                # then broadcast+normalize (norm must finish before the next
                # qb's PV reuses the psO banks)
                # evacuate both PV tiles first (frees the psO banks so the
                # next block's PV can't head-of-line block the PE queue),
                # then run the reciprocal chain from SBUF off-critical-path
                for h in heads:
                    hh = h % 2
                    ou = rcp.tile([65, 512], F32, tag=f"ou{hh}",
                                  name=f"ou{hh}", bufs=3)
                    nc.vector.tensor_copy(ou[:], pso[h][:])
                    ou_all[(hp, hh, qb)] = ou
                for h in heads:
                    hh = h % 2
                    dd = rcp.tile([1, 512], F32, tag=f"dd{hh}",
                                  name=f"dd{hh}", bufs=2)
                    nc.sync.dma_start(dd[:], ou_all[(hp, hh, qb)][64:65, :])
                    ddr = rcp.tile([1, 512], F32, tag=f"ddr{hh}",
                                   name=f"ddr{hh}", bufs=2)
                    nc.vector.reciprocal_approx_fast(ddr[:], dd[:])
                    dd_all[(hp, hh, qb)] = ddr
                for hh in (0, 1):
                    norm_emit(hp, hh, qb)
                if hp == 1:
                    # this block's projection is gated on the norm chain just
                    # emitted; inject it now so it drains during the NEXT
                    # block's attention instead of piling up at the end
                    inject += [proj_unit(tt, mb)
                               for tt in range(4 * qb, 4 * qb + 4)
                               for mb in (0, 1)]
                    if qb <= 1:
                        # warm-filler: the late norm chains idle the PE long
                        # enough to re-throttle the clock; these matmuls are
                        # WAR-gated behind this block's PV evacuation so they
                        # fill exactly the lull and keep the final projections
                        # at full rate
                        wps3 = psO.tile([64, 512], F32, tag="o0",
                                        name=f"warm3_{qb}")
                        for i in range(48 if qb == 1 else 144):
                            nc.tensor.matmul(
                                wps3[:, 0:64], ones_t[0:64, 0:64],
                                ones_t[0:64, 0:64], start=True, stop=True)
        # drain: the final block's projection
        while inject:
            inject.pop(0)()
        ctx.close()

    nc.finalize()
    return nc


def _pack(a, inner):
    """[1024, inner] -> [128, 8, inner] with [p, k, :] = a[k*128+p, :]."""
    return np.ascontiguousarray(
        a.reshape(NDC, 128, inner).transpose(1, 0, 2))


def shard_inputs(x, Wqkv, bqkv, Wproj):
    x = np.asarray(x, dtype=np.float32)
    Wqkv = np.asarray(Wqkv, dtype=np.float32)
    bqkv = np.asarray(bqkv, dtype=np.float32)
    Wproj = np.asarray(Wproj, dtype=np.float32)
    assert not np.any(bqkv[0:2048]), \
        "nonzero q/k bias not supported by the fast kernel"
    in_maps = []
    for c in range(8):
        b, g = c // 4, c % 4
        cs = slice(256 * g, 256 * g + 256)
        wq_ = Wqkv[:, 0:1024][:, cs] / 8.0
        wk_ = Wqkv[:, 1024:2048][:, cs]
        wqk_ = np.concatenate([wq_, wk_], axis=1)  # [1024, 512]
        wv_src = Wqkv[:, 2048:3072][:, cs]
        wv_ = np.zeros((DM, 260), dtype=np.float32)
        for h in range(4):
            wv_[:, 65 * h:65 * h + 64] = wv_src[:, 64 * h:64 * h + 64]
        wp_ = np.ascontiguousarray(
            Wproj[256 * g:256 * g + 256, :].reshape(2, 128, DM)
            .transpose(1, 0, 2))
        in_maps.append({
            "xp": _pack(x[b].T, T).astype(BF),
            "wqk": _pack(wqk_, 512).astype(BF),
            "wv": _pack(wv_, 260).astype(BF),
            "wp": wp_.astype(BF),
        })
    return in_maps


def combine_outputs(results, Wqkv, bqkv, Wproj, bproj):
    bqkv = np.asarray(bqkv, dtype=np.float32)
    Wproj = np.asarray(Wproj, dtype=np.float32)
    bproj = np.asarray(bproj, dtype=np.float32)
    bv_term = bqkv[2048:3072] @ Wproj
    out = np.zeros((2, T, DM), dtype=np.float32)
    for c in range(8):
        out[c // 4] += results[c]["y"].astype(np.float32).reshape(T, DM)
    out += (bv_term + bproj)[None, None, :]
    return out


_NC_CACHE = []


def _numpy_fallback(x, Wqkv, bqkv, Wproj, bproj):
    b, t, dm = x.shape
    h, d = 16, 64
    qkv = x @ Wqkv + bqkv
    q, k, v = np.split(qkv, 3, axis=-1)
    q = q.reshape(b, t, h, d).transpose(0, 2, 1, 3)
    k = k.reshape(b, t, h, d).transpose(0, 2, 1, 3)
    v = v.reshape(b, t, h, d).transpose(0, 2, 1, 3)
    att = np.einsum('bhqd,bhkd->bhqk', q, k) / np.sqrt(np.float32(d))
    causal = np.tril(np.ones((t, t), dtype=bool))
    att = np.where(causal[None, None], att, -np.inf)
    att = att - att.max(axis=-1, keepdims=True)
    e = np.exp(att)
    p = e / e.sum(axis=-1, keepdims=True)
    out = np.einsum('bhqk,bhkd->bhqd', p, v)
    out = out.transpose(0, 2, 1, 3).reshape(b, t, dm)
    return (out @ Wproj + bproj).astype(np.float32)


def kernel(x, Wqkv, bqkv, Wproj, bproj):
    x = np.asarray(x, dtype=np.float32)
    Wqkv = np.asarray(Wqkv, dtype=np.float32)
    bqkv = np.asarray(bqkv, dtype=np.float32)
    Wproj = np.asarray(Wproj, dtype=np.float32)
    bproj = np.asarray(bproj, dtype=np.float32)
    if np.any(bqkv[0:2048]):
        return _numpy_fallback(x, Wqkv, bqkv, Wproj, bproj)
    from concourse.bass_utils import run_bass_kernel_spmd
    if not _NC_CACHE:
        _NC_CACHE.append(build_nc())
    nc = _NC_CACHE[0]
    in_maps = shard_inputs(x, Wqkv, bqkv, Wproj)
    res = run_bass_kernel_spmd(nc, in_maps, core_ids=list(range(8)))
    return combine_outputs(res.results, Wqkv, bqkv, Wproj, bproj)
